# revision 11
# baseline (speedup 1.0000x reference)
"""2-layer GCN encoder as a distributed Bass kernel on 8 TRN2 NeuronCores.

Decomposition (per core, nodes sharded by destination):
  hs1[v] = dinv[v] * (x[v] @ W1)                  (own rows, AllGather, bf16)
  S1[d]  = sum_{e: dst=d} hs1[src_e]              (dma_gather + one-hot matmul)
  out1   = dinv*S1 + b1 ; hsr = dinv*relu(out1)   (own rows, AllGather, bf16)
  S2[d]  = sum_{e: dst=d} hsr[src_e]
  y[d]   = dinv[d]*(S2[d] @ W2) + b2              (W2 commutes with the sum)

The one-hot (0/1 structure only) is precomputed on host and streamed in as
bf16; all normalization (dinv = rsqrt(deg)) is computed and applied on
device as per-partition PSUM scales.  Edge gathers use SWDGE dma_gather
(<=1024 indices per instruction, round-robin over 4 SWDGE queues).
Self-loops are just extra edges.
"""

import numpy as np

import concourse.bass as bass
import concourse.bacc as bacc
import concourse.mybir as mybir
import concourse.tile as tile
from concourse import library_config
from concourse.bass_utils import run_bass_kernel_spmd

F32 = mybir.dt.float32
BF16 = mybir.dt.bfloat16
I16 = mybir.dt.int16

NCORES = 8
BLK = 128
# Max 128-index chunks per dma_gather instruction: the SWDGE descriptor
# ring holds only ~100 descriptors per DMA engine and a gather generates
# num_idxs/16 per ring; >=1792 indices hangs the ring-reclaim wait in the
# Q7 decode and wedges the device.  1024 is safe.
MAXCH = 8
NQUEUES = 4


def _cdiv(a, b):
    return (a + b - 1) // b


def preprocess(x, edge_index, ncores=NCORES):
    """Host-side graph partitioning: shard nodes/edges by dst, sort edges,
    build per-core gather indices (SWDGE wrapped layout), the 0/1 one-hot
    chunk matrices, and per-node degree counts.  Index/structure work only;
    all float math happens on device."""
    import ml_dtypes

    N, IN = x.shape
    assert N % ncores == 0
    NP = N // ncores
    nblk = _cdiv(NP, BLK)

    src = np.asarray(edge_index[0], dtype=np.int64)
    dst = np.asarray(edge_index[1], dtype=np.int64)
    # degree includes the self-loop, but self-loops are handled by an
    # identity matmul on device, not by the edge gather
    deg = (np.bincount(dst, minlength=N) + 1).astype(np.float32)

    # dedupe repeated (src, dst) pairs; multiplicity goes into the multi-hot
    key = dst * N + src
    ukey, mult = np.unique(key, return_counts=True)
    dst_s = ukey // N
    src_s = ukey % N

    bounds = np.array(
        [i * NP + b * BLK for i in range(ncores) for b in range(nblk)] + [N],
        dtype=np.int64,
    )
    pos = np.searchsorted(dst_s, bounds)
    cnt = np.diff(pos).reshape(ncores, nblk)

    # uniform chunk counts across cores (SPMD: one program for all cores)
    CH = np.maximum(1, _cdiv(cnt.max(axis=0), 128)).astype(np.int64)
    cofs = np.concatenate([[0], np.cumsum(CH)]).astype(np.int64)
    NCHT = int(CH.sum())
    widths = [min(BLK, NP - b * BLK) for b in range(nblk)]

    per_core = []
    for i in range(ncores):
        gidx = np.full((128, NCHT * 8), -1, np.int16)
        ohs = np.zeros((128, NCHT * 128), np.float32)
        nvalid = np.zeros(nblk, np.int64)
        for b in range(nblk):
            k = i * nblk + b
            s0, s1 = pos[k], pos[k + 1]
            esrc = src_s[s0:s1]
            edst = dst_s[s0:s1]
            ne = s1 - s0
            nvalid[b] = ne
            L = int(CH[b]) * 128
            idxp = np.concatenate([esrc, np.full(L - ne, -1, np.int64)]).astype(np.int16)
            wr = idxp.reshape(L // 16, 16).T  # [16, L//16]
            gidx[:, cofs[b] * 8 : cofs[b + 1] * 8] = np.tile(wr, (8, 1))
            # multi-hot: deduped edge at (chunk c, partition p) adds its
            # multiplicity into the local dst col
            dl = (edst - (i * NP + b * BLK)).astype(np.int64)  # [ne] in [0, w)
            e = np.arange(ne)
            c = e // 128
            p = e % 128
            np.add.at(ohs, (p, (cofs[b] + c) * 128 + dl), mult[s0:s1].astype(np.float32))
        degp = np.concatenate(
            [deg[i * NP : (i + 1) * NP], np.ones(nblk * BLK - NP, np.float32)]
        )
        per_core.append(
            {
                "x_tr": np.ascontiguousarray(x[i * NP : (i + 1) * NP].T),
                "deg_own": np.ascontiguousarray(degp.reshape(nblk, BLK).T),
                "gidx": gidx,
                "ohs": ohs.astype(ml_dtypes.bfloat16),
                "nvalid": nvalid,
            }
        )

    # make per-segment valid counts uniform across cores: num_idxs_reg is
    # baked into the shared program, and the ucode asserts it equals the
    # count of non-negative indices
    segs = []  # (block, seg_start_chunk, seg_chunks)
    for b in range(nblk):
        for s0 in range(0, int(CH[b]), 8):
            segs.append((b, s0, min(8, int(CH[b]) - s0)))
    seg_valid = []
    for (b, s0, sch) in segs:
        lo = s0 * 128
        hi = lo + sch * 128
        v = max(min(int(pc["nvalid"][b]) - lo, sch * 128) for pc in per_core)
        v = max(v, 1)
        seg_valid.append(v)
        for pc in per_core:
            have = min(max(int(pc["nvalid"][b]) - lo, 0), sch * 128)
            if have < v:
                # pad with index 0 at wrapped positions [have, v)
                kk = np.arange(lo + have, lo + v)
                cols = int(cofs[b]) * 8 + kk // 16
                rows = kk % 16
                for r in range(8):
                    pc["gidx"][16 * r + rows, cols] = 0
    for pc in per_core:
        del pc["nvalid"]

    meta = {
        "seg_valid": seg_valid,
        "N": N,
        "NP": NP,
        "IN": IN,
        "nblk": nblk,
        "CH": [int(c) for c in CH],
        "cofs": [int(c) for c in cofs],
        "widths": widths,
        "NCHT": NCHT,
    }
    return per_core, meta


def build_nc(meta, HID, OUT, ncores=NCORES):
    N, NP, IN = meta["N"], meta["NP"], meta["IN"]
    nblk, CH, cofs, widths = meta["nblk"], meta["CH"], meta["cofs"], meta["widths"]
    seg_valid = meta["seg_valid"]
    NCHT = meta["NCHT"]
    KC = IN // 128
    assert IN % 128 == 0 and HID == 128 and OUT <= 512

    nc = bacc.Bacc(
        "TRN2",
        target_bir_lowering=False,
        debug=False,
        num_devices=ncores,
        num_swdge_queues=NQUEUES,
    )

    x_tr = nc.dram_tensor("x_tr", [IN, NP], F32, kind="ExternalInput")
    w1 = nc.dram_tensor("w1", [IN, HID], F32, kind="ExternalInput")
    b1 = nc.dram_tensor("b1", [1, HID], F32, kind="ExternalInput")
    w2 = nc.dram_tensor("w2", [HID, OUT], F32, kind="ExternalInput")
    b2 = nc.dram_tensor("b2", [1, OUT], F32, kind="ExternalInput")
    deg_own = nc.dram_tensor("deg_own", [128, nblk], F32, kind="ExternalInput")
    gidx_d = nc.dram_tensor("gidx", [128, NCHT * 8], I16, kind="ExternalInput")
    ident_d = nc.dram_tensor("ident", [128, 128], BF16, kind="ExternalInput")
    ohs_d = nc.dram_tensor("ohs", [128, NCHT * 128], BF16, kind="ExternalInput")
    y = nc.dram_tensor("y", [NP, OUT], F32, kind="ExternalOutput")

    hs1_stage = nc.dram_tensor("hs1_stage", [NP, HID], BF16)
    hs1_full = nc.dram_tensor("hs1_full", [N, HID], BF16, addr_space="Shared")
    hsr_stage = nc.dram_tensor("hsr_stage", [NP, HID], BF16)
    hsr_full = nc.dram_tensor("hsr_full", [N, HID], BF16, addr_space="Shared")

    rg = [list(range(ncores))]
    qn = [0]

    def next_q():
        q = qn[0]
        qn[0] = (q + 1) % NQUEUES
        return q

    with tile.TileContext(nc) as tc:
        nc.gpsimd.load_library(library_config.mlp)
        with (
            tc.tile_pool(name="const", bufs=1) as constp,
            tc.tile_pool(name="gath", bufs=8) as gathp,
            tc.tile_pool(name="oh", bufs=6) as ohp,
            tc.tile_pool(name="hs", bufs=4) as hsp,
            tc.tile_pool(name="ps", bufs=3, space="PSUM") as psp,
            tc.tile_pool(name="pso", bufs=2, space="PSUM") as psop,
        ):
            # ---- constants ----
            w1c = []
            for k in range(KC):
                t = constp.tile([128, HID], BF16, tag=f"w1c{k}")
                nc.gpsimd.dma_start(out=t[:], in_=w1[k * 128 : (k + 1) * 128, :])
                w1c.append(t)
            w2_sb = constp.tile([HID, OUT], F32, tag="w2")
            nc.sync.dma_start(out=w2_sb[:], in_=w2[:, :])
            b1_sb = constp.tile([1, HID], F32, tag="b1")
            nc.sync.dma_start(out=b1_sb[:], in_=b1[:, :])
            b2_sb = constp.tile([1, OUT], F32, tag="b2")
            nc.sync.dma_start(out=b2_sb[:], in_=b2[:, :])
            ones_sb = constp.tile([1, 128], F32, tag="ones")
            nc.vector.memset(ones_sb[:], 1.0)

            # broadcast b1/b2 to all partitions via rank-1 matmul
            pb = psop.tile([128, HID], F32, tag="po")
            nc.tensor.matmul(pb[:], lhsT=ones_sb[:], rhs=b1_sb[:],
                             start=True, stop=True)
            b1_bc = constp.tile([128, HID], F32, tag="b1bc")
            nc.vector.tensor_copy(b1_bc[:], pb[:])
            pb2 = psop.tile([128, OUT], F32, tag="po")
            nc.tensor.matmul(pb2[:], lhsT=ones_sb[:], rhs=b2_sb[:],
                             start=True, stop=True)
            b2_bc = constp.tile([128, OUT], F32, tag="b2bc")
            nc.vector.tensor_copy(b2_bc[:], pb2[:])

            dinv_sb = constp.tile([128, nblk], F32, tag="dinv")
            nc.sync.dma_start(out=dinv_sb[:], in_=deg_own[:, :])
            nc.scalar.sqrt(dinv_sb[:], dinv_sb[:])
            nc.vector.reciprocal(dinv_sb[:], dinv_sb[:])

            gidx_sb = constp.tile([128, NCHT * 8], I16, tag="gidx")
            nc.sync.dma_start(out=gidx_sb[:], in_=gidx_d[:, :])
            ident_sb = constp.tile([128, 128], BF16, tag="ident")
            nc.sync.dma_start(out=ident_sb[:], in_=ident_d[:, :])

            xsb = []
            for k in range(KC):
                t = constp.tile([128, NP], BF16, tag=f"x{k}")
                nc.gpsimd.dma_start(out=t[:], in_=x_tr[k * 128 : (k + 1) * 128, :])
                xsb.append(t)

            # ---- phase B: hs1 = dinv * (x @ W1) for own rows ----
            for b in range(nblk):
                w = widths[b]
                ph = psp.tile([128, HID], F32, tag="acc")
                for k in range(KC):
                    nc.tensor.matmul(
                        ph[:w, :],
                        lhsT=xsb[k][:, b * BLK : b * BLK + w],
                        rhs=w1c[k][:, :],
                        start=(k == 0),
                        stop=(k == KC - 1),
                    )
                hs1_t = hsp.tile([128, HID], BF16, tag="hs1")
                nc.scalar.activation(
                    hs1_t[:w, :],
                    ph[:w, :],
                    mybir.ActivationFunctionType.Copy,
                    scale=dinv_sb[:w, b : b + 1],
                )
                nc.sync.dma_start(
                    out=hs1_stage[b * BLK : b * BLK + w, :], in_=hs1_t[:w, :]
                )

            # ---- AllGather 1 ----
            nc.gpsimd.collective_compute(
                "AllGather",
                mybir.AluOpType.bypass,
                replica_groups=rg,
                ins=[hs1_stage.ap().opt()],
                outs=[hs1_full.ap().opt()],
            )

            # ---- phase D: S1 -> out1 -> hsr (own dst blocks) ----
            segi = [0]
            for b in range(nblk):
                w = widths[b]
                ch = CH[b]
                c0 = cofs[b]
                p1 = psp.tile([128, HID], F32, tag="acc")
                # self-loop: S1 += hs1[own block] via identity matmul
                hsl1 = hsp.tile([128, HID], BF16, tag="hsl")
                nc.sync.dma_start(
                    out=hsl1[:w, :], in_=hs1_stage[b * BLK : b * BLK + w, :]
                )
                nc.tensor.matmul(
                    p1[:w, :], lhsT=ident_sb[:w, :w], rhs=hsl1[:w, :],
                    start=True, stop=False,
                )
                for s0 in range(0, ch, MAXCH):
                    sch = min(MAXCH, ch - s0)
                    nv = seg_valid[segi[0]]
                    segi[0] += 1
                    g1 = gathp.tile([128, MAXCH, HID], BF16, tag="g")
                    nc.gpsimd.dma_gather(
                        g1[:, :sch, :],
                        hs1_full.ap(),
                        gidx_sb[:, (c0 + s0) * 8 : (c0 + s0 + sch) * 8],
                        sch * 128,
                        nv,
                        HID,
                        queue_num=next_q(),
                    )
                    ohc = ohp.tile([128, MAXCH * 128], BF16, tag="oh")
                    nc.sync.dma_start(
                        out=ohc[:, : sch * 128],
                        in_=ohs_d[:, (c0 + s0) * 128 : (c0 + s0 + sch) * 128],
                    )
                    for c in range(sch):
                        vk = min(128, nv - c * 128)
                        nc.tensor.matmul(
                            p1[:w, :],
                            lhsT=ohc[:vk, c * 128 : c * 128 + w],
                            rhs=g1[:vk, c, :],
                            start=False,
                            stop=(s0 + c == ch - 1),
                        )
                # out1 = dinv*S1 + b1 ; hsr = dinv*relu(out1)
                t1 = hsp.tile([128, HID], F32, tag="t1")
                nc.scalar.activation(
                    t1[:w, :], p1[:w, :],
                    mybir.ActivationFunctionType.Copy,
                    scale=dinv_sb[:w, b : b + 1],
                )
                t2 = hsp.tile([128, HID], F32, tag="t2")
                nc.vector.tensor_tensor(
                    out=t2[:w, :], in0=t1[:w, :], in1=b1_bc[:w, :],
                    op=mybir.AluOpType.add,
                )
                hsr_t = hsp.tile([128, HID], BF16, tag="hsr")
                nc.scalar.activation(
                    hsr_t[:w, :], t2[:w, :],
                    mybir.ActivationFunctionType.Relu,
                    scale=dinv_sb[:w, b : b + 1],
                )
                nc.sync.dma_start(
                    out=hsr_stage[b * BLK : b * BLK + w, :], in_=hsr_t[:w, :]
                )

            # ---- AllGather 2 ----
            nc.gpsimd.collective_compute(
                "AllGather",
                mybir.AluOpType.bypass,
                replica_groups=rg,
                ins=[hsr_stage.ap().opt()],
                outs=[hsr_full.ap().opt()],
            )

            # ---- phase F: S2 -> y ----
            segi[0] = 0
            for b in range(nblk):
                w = widths[b]
                ch = CH[b]
                c0 = cofs[b]
                p2 = psp.tile([128, 128], F32, tag="acc")
                # self-loop: S2^T += hsr[own block]^T via identity matmul
                hsl2 = hsp.tile([128, HID], BF16, tag="hsl")
                nc.sync.dma_start(
                    out=hsl2[:w, :], in_=hsr_stage[b * BLK : b * BLK + w, :]
                )
                nc.tensor.matmul(
                    p2[:, :w], lhsT=hsl2[:w, :], rhs=ident_sb[:w, :w],
                    start=True, stop=False,
                )
                for s0 in range(0, ch, MAXCH):
                    sch = min(MAXCH, ch - s0)
                    nv = seg_valid[segi[0]]
                    segi[0] += 1
                    g2 = gathp.tile([128, MAXCH, HID], BF16, tag="g")
                    nc.gpsimd.dma_gather(
                        g2[:, :sch, :],
                        hsr_full.ap(),
                        gidx_sb[:, (c0 + s0) * 8 : (c0 + s0 + sch) * 8],
                        sch * 128,
                        nv,
                        HID,
                        queue_num=next_q(),
                    )
                    ohc = ohp.tile([128, MAXCH * 128], BF16, tag="oh")
                    nc.sync.dma_start(
                        out=ohc[:, : sch * 128],
                        in_=ohs_d[:, (c0 + s0) * 128 : (c0 + s0 + sch) * 128],
                    )
                    for c in range(sch):
                        vk = min(128, nv - c * 128)
                        nc.tensor.matmul(
                            p2[:, :w],
                            lhsT=g2[:vk, c, :],
                            rhs=ohc[:vk, c * 128 : c * 128 + w],
                            start=False,
                            stop=(s0 + c == ch - 1),
                        )
                # aggT [128, w] = S2^T ; y = dinv*(S2@W2) + b2
                aggT = hsp.tile([128, 128], F32, tag="aggT")
                nc.scalar.activation(
                    aggT[:, :w], p2[:, :w], mybir.ActivationFunctionType.Copy
                )
                po = psop.tile([128, OUT], F32, tag="po")
                nc.tensor.matmul(
                    po[:w, :], lhsT=aggT[:, :w], rhs=w2_sb[:, :],
                    start=True, stop=True,
                )
                o1 = hsp.tile([128, OUT], F32, tag="o1")
                nc.scalar.activation(
                    o1[:w, :], po[:w, :],
                    mybir.ActivationFunctionType.Copy,
                    scale=dinv_sb[:w, b : b + 1],
                )
                yt = hsp.tile([128, OUT], F32, tag="yt")
                nc.vector.tensor_tensor(
                    out=yt[:w, :], in0=o1[:w, :], in1=b2_bc[:w, :],
                    op=mybir.AluOpType.add,
                )
                nc.sync.dma_start(out=y[b * BLK : b * BLK + w, :], in_=yt[:w, :])

    nc.compile()
    return nc


def _make_ident():
    import ml_dtypes

    return np.eye(128, dtype=np.float32).astype(ml_dtypes.bfloat16)


_IDENT = _make_ident()


def make_in_maps(per_core, W1, b1, W2, b2):
    W1 = np.ascontiguousarray(np.asarray(W1, np.float32))
    W2 = np.ascontiguousarray(np.asarray(W2, np.float32))
    b1 = np.asarray(b1, np.float32).reshape(1, -1)
    b2 = np.asarray(b2, np.float32).reshape(1, -1)
    return [
        {
            "x_tr": pc["x_tr"],
            "w1": W1,
            "b1": b1,
            "w2": W2,
            "b2": b2,
            "deg_own": pc["deg_own"],
            "gidx": pc["gidx"],
            "ohs": pc["ohs"],
            "ident": _IDENT,
        }
        for pc in per_core
    ]


def kernel_run(x, edge_index, W1, b1, W2, b2, trace=False):
    x = np.ascontiguousarray(np.asarray(x, np.float32))
    per_core, meta = preprocess(x, edge_index)
    HID = np.asarray(W1).shape[1]
    OUT = np.asarray(W2).shape[1]
    nc = build_nc(meta, HID, OUT)
    in_maps = make_in_maps(per_core, W1, b1, W2, b2)
    res = run_bass_kernel_spmd(nc, in_maps, core_ids=list(range(NCORES)), trace=trace)
    out = np.concatenate([r["y"] for r in res.results], axis=0)
    return out, res


def kernel(x, edge_index, W1, b1, W2, b2):
    out, _ = kernel_run(x, edge_index, W1, b1, W2, b2)
    return out
